# revision 31
# baseline (speedup 1.0000x reference)
"""CRF loss kernel for Trainium2 (8 NeuronCores, data-parallel over batch).

Math: loss = sum_b logZ_b - sum_b gold_b   (lengths unused by the reference).

Forward algorithm in the exp domain:
    P_t = D_t E P_{t-1},  D_t = diag(exp(feats[:, t-1, :])),  E = exp(transitions)
    logZ = ln(estop^T P_T),  estop = exp(transitions[STOP, :])
Run half the time steps forward (P chain) and half backward
(gamma_t = F_t o (E^T gamma_{t+1}), gamma_512 = F_512 o estop), meeting at T/2:
    logZ = ln(beta_256^T P_256),  beta_256 = E^T gamma_257.
Each E application is pre-scaled by exp(-c0) (c0 ~ mean per-step log-growth,
estimated on host).  With that centering the chain magnitude drift stays
within e^{+-8} for this data (measured), well inside bf16/f32 range, so NO
renormalization steps are needed at all.

Gold score on the tensor engine via host-built one-hot matrices, with fp8
DoubleRow matmuls (two 128-row chunks contracted per instruction):
    emit  = trace( sum_chunks OHc^T @ feats_chunk )
    trans = < sum_chunks OHc^T @ OHp , transitions >
with an extra row per example for the STOP transition.
"""

import os
import sys

sys.path.insert(0, "/opt/trn_rl_repo")

import numpy as np
import ml_dtypes

import concourse.bass as bass
import concourse.tile as tile
from concourse import mybir
from concourse.bass_utils import run_bass_kernel_spmd

B, T, K = 512, 512, 128
NCORES = 8
BL = B // NCORES
START, STOP = 126, 127
HALF = T // 2
FCH = 16  # time steps per F chunk
NFCH = HALF // FCH  # chunks per stream
GROWS = 33280  # BL*T + BL stop rows, padded to 130*256
NPAIR = GROWS // 256  # 130 DoubleRow pairs
GJ = 10  # pairs per DMA group
NGDMA = NPAIR // GJ  # 13 dma groups

bf16 = mybir.dt.bfloat16
f32 = mybir.dt.float32
fp8 = mybir.dt.float8e4
NP_BF16 = np.dtype(ml_dtypes.bfloat16)
NP_FP8 = np.dtype(mybir.dt.np(fp8))

_cached = {}


def _fix_multiwait(nc):
    """Walrus here accepts a single sync-wait per instruction; hoist extra
    waits onto single-wait NoOps inserted before the offender.

    Wait choice matters for latency: the wait kept ON the instruction should
    be the one most likely to actually block (a cross-engine data dep), so
    the hoisted NoOps retire early and add no serial hop.  Same-engine
    ge-waits are trivially satisfied (in-order completion, monotone sems)
    and DMA-queue waits are prefetch-slack — hoist those.
    """
    # sem id -> set of engines whose instructions update it
    sem_engines = {}
    for f in nc.m.functions:
        for bb in f.blocks:
            for inst in bb.instructions:
                si = getattr(inst, "sync_info", None)
                if si is None:
                    continue
                for u in si.on_update:
                    uid = getattr(u, "id", None)
                    if uid is not None:
                        sem_engines.setdefault(uid, set()).add(inst.engine)

    n = 0
    for f in nc.m.functions:
        for bb in f.blocks:
            insts = bb.instructions
            out = []
            changed = False
            for inst in insts:
                si = getattr(inst, "sync_info", None)
                if si is not None and len(si.on_wait) > 1:
                    # merge redundant ge-waits on the same semaphore
                    merged = {}
                    rest = []
                    for w in si.on_wait:
                        if getattr(w, "wait_mode", None) == "sem-ge-imm":
                            key = w.id
                            if key in merged:
                                if w.wait_value > merged[key].wait_value:
                                    merged[key] = w
                            else:
                                merged[key] = w
                        else:
                            rest.append(w)
                    waits = list(merged.values()) + rest

                    def prio(w):
                        engs = sem_engines.get(getattr(w, "id", None))
                        if engs is None:
                            return 0  # DMA/external: prefetched, hoist first
                        if engs == {inst.engine}:
                            return 1  # self-engine: trivially satisfied
                        return 2  # cross-engine data dep: keep on inst

                    # a ge-wait on a sem only ever incremented by THIS engine
                    # is satisfied by in-order execution; drop it entirely
                    waits = [
                        w
                        for w in waits
                        if not (
                            prio(w) == 1
                            and getattr(w, "wait_mode", None) == "sem-ge-imm"
                        )
                    ] or waits[-1:]
                    waits.sort(key=prio)
                    if len(waits) == 1:
                        inst.sync_info = mybir.SyncInfo(
                            on_wait=waits, on_update=list(si.on_update)
                        )
                        out.append(inst)
                        continue
                    for j, w in enumerate(waits[:-1]):
                        out.append(
                            mybir.InstNoOp(
                                name=f"{inst.name}-ws{j}",
                                engine=inst.engine,
                                sync_info=mybir.SyncInfo(
                                    on_wait=[w], on_update=[]
                                ),
                                bass_nofuse=True,
                            )
                        )
                        n += 1
                    inst.sync_info = mybir.SyncInfo(
                        on_wait=[waits[-1]], on_update=list(si.on_update)
                    )
                    changed = True
                out.append(inst)
            if changed:
                bb.instructions = out
    return n


def _build_module():
    from contextlib import ExitStack

    nc = bass.Bass("TRN2", target_bir_lowering=False, debug=False)

    def din(name, shape, dt):
        return nc.dram_tensor(name, shape, dt, kind="ExternalInput").ap()

    efwd = din("efwd", [K, K], bf16)  # lhsT for P-chain: exp(trans-c0).T
    ebwd = din("ebwd", [K, K], bf16)  # lhsT for gamma-chain: exp(trans-c0)
    zerosb = din("zerosb", [K, K], bf16)
    estop = din("estop", [K, 1], f32)
    p0 = din("p0", [K, BL], bf16)
    fkb = din("fkb", [K, T, BL], bf16)  # feats, k-major
    grhs = din("grhs", [GROWS, 2 * K], fp8)  # [feats | onehot(prev)] rows
    ohc = din("ohc", [GROWS, K], fp8)  # onehot(cur tag)
    onesb = din("onesb", [K, K], bf16)
    onesf = din("onesf", [K, K], f32)
    ident = din("ident", [K, K], f32)
    transf = din("transf", [K, K], f32)
    out_ap = nc.dram_tensor("out", [1, 2], f32, kind="ExternalOutput").ap()

    # DoubleRow pair layout: pair j = chunks (2j, 2j+1); chunk i is rows
    # [ (g*GJ+j)*256 + i*128 + p ] of the row-major DRAM tensors.
    grhs_g = grhs.rearrange("(g j i p) n -> g p j i n", p=128, i=2, j=GJ)
    ohc_g = ohc.rearrange("(g j i p) k -> g p j i k", p=128, i=2, j=GJ)

    AL = mybir.AluOpType

    with tile.TileContext(nc) as tc:
        with ExitStack() as ctx:
            consts = ctx.enter_context(tc.tile_pool(name="consts", bufs=1))
            state = ctx.enter_context(tc.tile_pool(name="state", bufs=4))
            fraw = ctx.enter_context(tc.tile_pool(name="fraw", bufs=3))
            fexp = ctx.enter_context(tc.tile_pool(name="fexp", bufs=3))
            goldp = ctx.enter_context(tc.tile_pool(name="goldp", bufs=3))
            smalls = ctx.enter_context(tc.tile_pool(name="smalls", bufs=4))
            psf = ctx.enter_context(
                tc.tile_pool(name="psf", bufs=3, space="PSUM")
            )
            psb = ctx.enter_context(
                tc.tile_pool(name="psb", bufs=3, space="PSUM")
            )
            psj = ctx.enter_context(
                tc.tile_pool(name="psj", bufs=1, space="PSUM")
            )
            psacc = ctx.enter_context(
                tc.tile_pool(name="psacc", bufs=1, space="PSUM")
            )

            # ---- chain-critical constants first (single sync DMA queue) ----
            efwd_sb = consts.tile([K, K], bf16)
            nc.sync.dma_start(efwd_sb[:], efwd[:, :])
            ebwd_sb = consts.tile([K, K], bf16)
            estop_sb = consts.tile([K, 1], f32)
            zerosb_sb = consts.tile([K, K], bf16)
            nc.gpsimd.memset(zerosb_sb[:], 0.0)

            def load_start_consts2():
                nc.sync.dma_start(estop_sb[:], estop[:, :])
                nc.sync.dma_start(ebwd_sb[:], ebwd[:, :])
            # junction-only constants are DMA'd after the loop is underway
            onesb_sb = consts.tile([K, K], bf16)
            onesf_sb = consts.tile([K, K], f32)
            ident_sb = consts.tile([K, K], f32)
            transf_sb = consts.tile([K, K], f32)

            def load_tail_consts():
                nc.sync.dma_start(onesb_sb[:], onesb[:, :])
                nc.sync.dma_start(onesf_sb[:], onesf[:, :])
                nc.sync.dma_start(ident_sb[:], ident[:, :])
                nc.sync.dma_start(transf_sb[:], transf[:, :])

            # gold PSUM accumulator: [OHc^T @ feats | OHc^T @ OHp]
            a12 = psacc.tile([K, 2 * K], f32)

            # ---- F chunk machinery ----
            # Irregular chunk lengths: a tiny first chunk gets the chains
            # started ~8us earlier; steady-state chunks are FCH steps.
            CLENS = [4] + [FCH] * 15 + [HALF - 4 - 15 * FCH]
            CSTARTS = [sum(CLENS[:i]) for i in range(len(CLENS))]
            NCH = len(CLENS)
            ftiles = [{}, {}]

            def ensure_fchunk(stream, c):
                if c >= NCH or c in ftiles[stream]:
                    return
                # chunk c covers chain offsets [CSTARTS[c], +CLENS[c]) of
                # stream: fwd feats idx o; bwd feats idx T-1-o
                ln = CLENS[c]
                t0 = (
                    CSTARTS[c]
                    if stream == 0
                    else T - CSTARTS[c] - ln
                )
                raw = fraw.tile([K, ln, BL], bf16, tag=f"raw{stream}")
                nc.sync.dma_start(raw[:], fkb[:, t0 : t0 + ln, :])
                fe = fexp.tile([K, ln, BL], bf16, tag=f"fe{stream}")
                nc.scalar.activation(
                    fe[:], raw[:], mybir.ActivationFunctionType.Exp
                )
                ftiles[stream][c] = fe

            import bisect

            def chunk_of(o):
                return bisect.bisect_right(CSTARTS, o) - 1

            def fslice(stream, fi):
                # fi is a feats index; stream offset o: fwd o=fi, bwd o=T-1-fi
                o = fi if stream == 0 else T - 1 - fi
                c = chunk_of(o)
                fe = ftiles[stream][c]
                off = o - CSTARTS[c]
                if stream == 1:
                    # bwd tile holds feats idx [T-CSTARTS-ln, T-CSTARTS) in
                    # ascending t; offset o counts DOWN from idx T-1
                    off = CLENS[c] - 1 - off
                return fe[:, off, :]

            ensure_fchunk(0, 0)
            ensure_fchunk(1, 0)

            # ---- chain state init ----
            p0_sb = consts.tile([K, BL], bf16)
            nc.sync.dma_start(p0_sb[:], p0[:, :])
            load_start_consts2()
            p_t = state.tile([K, BL], bf16, tag="P")
            nc.vector.tensor_copy(p_t[:], p0_sb[:])
            g_t = state.tile([K, BL], bf16, tag="G")
            # gamma_512 = F(feats idx 511) o estop (per-partition scalar)
            nc.vector.tensor_scalar_mul(g_t[:], fslice(1, T - 1), estop_sb[:])

            # ---- gold machinery (fp8 DoubleRow pairs) ----
            gold_tiles = {}

            def gold_load(g):
                if g >= NGDMA or g in gold_tiles:
                    return
                rh_t = goldp.tile([128, GJ, 2, 2 * K], fp8, tag="rh")
                nc.gpsimd.dma_start(rh_t[:], grhs_g[g])
                oc_t = goldp.tile([128, GJ, 2, K], fp8, tag="oc")
                nc.gpsimd.dma_start(oc_t[:], ohc_g[g])
                gold_tiles[g] = (rh_t, oc_t)

            def gold_pair(pj):
                g, j = divmod(pj, GJ)
                rh_t, oc_t = gold_tiles[g]
                nc.tensor.matmul(
                    a12[:],
                    oc_t[:, j, :, :],
                    rh_t[:, j, :, :],
                    start=(pj == 0),
                    stop=(pj == NPAIR - 1),
                    perf_mode=mybir.MatmulPerfMode.DoubleRow,
                )

            gold_load(0)
            gold_load(1)

            def pace_mm(src):
                # zero-contribution accumulate into a12: numerically exact
                # (stationary is all zeros), but it (a) reads this round's
                # fresh chain state, and (b) sits in a12's program-order
                # write chain — so the statically-scheduled gold DR matmuls
                # behind it cannot clump ahead of the chain.  Doubles as a
                # PE-pipeline warmer between dependent chain matmuls.
                nc.tensor.matmul(
                    a12[:, 0:BL],
                    zerosb_sb[:],
                    src[:],
                    start=False,
                    stop=False,
                    skip_group_check=True,
                )

            # ---- main loop: 256 rounds, no renorms ----
            # Tail gold pairs (128..135) ride the odd rounds near the end;
            # their DMA groups are loaded by the in-loop prefetch.
            TAIL0 = HALF - 2 * (NPAIR - HALF // 2) + 1  # first tail round
            braw = None
            prev_p, prev_g = p0_sb, p0_sb
            for r in range(HALF):
                # fwd step r+1 (feats idx r)
                praw = psf.tile([K, BL], f32, tag="praw")
                nc.tensor.matmul(
                    praw[:], efwd_sb[:], p_t[:], start=True, stop=True
                )
                # bwd step (feats idx 510-r); at r=255 this matmul IS the
                # junction product beta_256 = E'^T gamma_257
                graw = psb.tile([K, BL], f32, tag="graw")
                nc.tensor.matmul(
                    graw[:], ebwd_sb[:], g_t[:], start=True, stop=True
                )
                # gold pair on even rounds; tail pairs on late odd rounds;
                # warm filler (paced on the PREVIOUS round's state, so it is
                # ready-to-run here and bridges the PE gap while this
                # round's TTs are still in flight) otherwise
                if r % 2 == 0 and r // 2 < NPAIR:
                    gold_pair(r // 2)
                elif r % 2 == 1 and r >= TAIL0:
                    gold_pair(HALF // 2 + (r - TAIL0) // 2)
                if r < HALF - 2:
                    pace_mm(prev_p)
                # paced filler reading the PREVIOUS round's state: already
                # runnable when it reaches the head of the PE stream, so it
                # bridges the gap while this round's TTs are in flight
                if r < HALF - 2:
                    pace_mm(prev_g)

                p_new = state.tile([K, BL], bf16, tag="P")
                nc.vector.tensor_tensor(
                    out=p_new[:], in0=praw[:], in1=fslice(0, r), op=AL.mult
                )
                prev_p = p_t
                p_t = p_new
                if r < HALF - 1:
                    g_new = state.tile([K, BL], bf16, tag="G")
                    nc.vector.tensor_tensor(
                        out=g_new[:],
                        in0=graw[:],
                        in1=fslice(1, T - 2 - r),
                        op=AL.mult,
                    )
                    prev_g = g_t
                    g_t = g_new
                else:
                    braw = graw

                if r == 2:
                    load_tail_consts()
                # prefetches, early in each chunk/group window
                c_now = chunk_of(r)
                if r == CSTARTS[c_now] + 1:
                    ensure_fchunk(0, c_now + 1)
                    ensure_fchunk(1, c_now + 1)
                    if c_now == 0:
                        ensure_fchunk(0, 2)
                        ensure_fchunk(1, 2)
                if r % 20 == 3:
                    gold_load(r // 20 + 1)
                if r % 20 == 11:
                    gold_load(r // 20 + 2)

            # ---- junction: J_b = sum_k braw[k,b] * P_256[k,b] ----
            jprod = smalls.tile([K, BL], bf16, tag="jprod")
            nc.vector.tensor_tensor(
                out=jprod[:], in0=braw[:], in1=p_t[:], op=AL.mult
            )
            jall_ps = psj.tile([K, BL], f32, tag="zps")
            nc.tensor.matmul(
                jall_ps[:], onesb_sb[:], jprod[:], start=True, stop=True
            )
            lnj = smalls.tile([1, BL], f32, tag="lnj")
            nc.scalar.activation(
                lnj[:], jall_ps[0:1, :], mybir.ActivationFunctionType.Ln
            )
            fwdsum = smalls.tile([1, 1], f32, tag="fwdsum")
            nc.vector.tensor_reduce(
                fwdsum[:], lnj[:], axis=mybir.AxisListType.X, op=AL.add
            )

            # ---- gold finals ----
            junk1 = smalls.tile([K, K], f32, tag="junk1")
            emit_pp = smalls.tile([K, 1], f32, tag="emit_pp")
            nc.vector.scalar_tensor_tensor(
                out=junk1[:],
                in0=a12[:, 0:K],
                scalar=1.0,
                in1=ident_sb[:],
                op0=AL.mult,
                op1=AL.mult,
                accum_out=emit_pp[:],
            )
            junk2 = smalls.tile([K, K], f32, tag="junk2")
            tr_pp = smalls.tile([K, 1], f32, tag="tr_pp")
            nc.vector.scalar_tensor_tensor(
                out=junk2[:],
                in0=a12[:, K : 2 * K],
                scalar=1.0,
                in1=transf_sb[:],
                op0=AL.mult,
                op1=AL.mult,
                accum_out=tr_pp[:],
            )
            gold_pp = smalls.tile([K, 1], bf16, tag="gold_pp")
            with nc.allow_low_precision(
                reason="per-tag gold partial sums; 0.4% rounding is far "
                "inside the 2e-2 tolerance"
            ):
                nc.vector.tensor_add(gold_pp[:], emit_pp[:], tr_pp[:])
            gall_ps = psj.tile([K, 1], f32, tag="zps")
            nc.tensor.matmul(
                gall_ps[:], onesb_sb[:], gold_pp[:], start=True, stop=True
            )

            # ---- output ----
            res = smalls.tile([1, 2], f32, tag="res")
            nc.vector.tensor_copy(res[:, 0:1], fwdsum[:])
            nc.vector.tensor_copy(res[:, 1:2], gall_ps[0:1, :])
            nc.sync.dma_start(out_ap[:, :], res[:])

    _fix_multiwait(nc)
    return nc


def _estimate_c0(feats, transitions):
    """Mean per-step log-growth of the forward recursion, from a few batches."""
    nb = 4
    E = np.exp(transitions.astype(np.float64))
    P = np.zeros((K, nb))
    P[START, :] = 1.0
    tot = 0.0
    for t in range(T):
        P = E @ P
        P = P * np.exp(feats[:nb, t, :].astype(np.float64)).T
        s = P.sum(axis=0)
        tot += np.log(s).mean()
        P /= s
    return tot / T


def _host_prep(feats, tags, transitions):
    c0 = _estimate_c0(feats, transitions)
    ep = np.exp(transitions.astype(np.float64) - c0)
    efwd_np = np.ascontiguousarray(ep.T).astype(NP_BF16)
    ebwd_np = np.ascontiguousarray(ep).astype(NP_BF16)
    estop_np = np.exp(transitions[STOP, :].astype(np.float64)).astype(
        np.float32
    )[:, None]
    ident_np = np.eye(K, dtype=np.float32)
    zerosb_np = np.zeros((K, K), dtype=NP_BF16)
    onesb_np = np.ones((K, K), dtype=NP_BF16)
    onesf_np = np.ones((K, K), dtype=np.float32)
    transf_np = transitions.astype(np.float32)
    p0_np = np.zeros((K, BL), dtype=NP_BF16)
    p0_np[START, :] = 1.0

    in_maps = []
    for c in range(NCORES):
        b0 = c * BL
        fc = feats[b0 : b0 + BL]  # [BL, T, K] f32
        tg = tags[b0 : b0 + BL].astype(np.int32)  # [BL, T]

        fkb_np = np.ascontiguousarray(fc.transpose(2, 1, 0)).astype(NP_BF16)

        nrow = BL * T
        grhs_np = np.zeros((GROWS, 2 * K), dtype=NP_FP8)
        grhs_np[:nrow, :K] = fc.reshape(nrow, K).astype(NP_FP8)
        ohc_np = np.zeros((GROWS, K), dtype=NP_FP8)
        rows = np.arange(nrow)
        ohc_np[rows, tg.reshape(nrow)] = 1.0
        prev = np.concatenate(
            [np.full((BL, 1), START, np.int32), tg[:, :-1]], axis=1
        )
        grhs_np[rows, K + prev.reshape(nrow)] = 1.0
        # stop rows: trans[STOP, tag_last] per example
        srows = nrow + np.arange(BL)
        ohc_np[srows, STOP] = 1.0
        grhs_np[srows, K + tg[:, -1]] = 1.0

        in_maps.append(
            {
                "efwd": efwd_np,
                "ebwd": ebwd_np,
                "zerosb": zerosb_np,
                "estop": estop_np,
                "p0": p0_np,
                "fkb": fkb_np,
                "grhs": grhs_np,
                "ohc": ohc_np,
                "ident": ident_np,
                "onesb": onesb_np,
                "onesf": onesf_np,
                "transf": transf_np,
            }
        )
    return in_maps, c0


last_exec_time_ns = None
last_results = None


def kernel(feats, tags, lengths, transitions):
    global last_exec_time_ns, last_results
    feats = np.asarray(feats, dtype=np.float32)
    tags = np.asarray(tags)
    transitions = np.asarray(transitions, dtype=np.float32)

    if "nc" not in _cached:
        _cached["nc"] = _build_module()
    nc = _cached["nc"]

    in_maps, c0 = _host_prep(feats, tags, transitions)

    trace = bool(int(os.environ.get("BASS_CRF_TRACE", "0")))
    kwargs = {}
    if trace:
        kwargs = {
            "trace": True,
            "tmpdir": os.environ.get("BASS_CRF_TMPDIR", "/tmp/crf_trace"),
        }
    res = run_bass_kernel_spmd(
        nc, in_maps, core_ids=list(range(NCORES)), **kwargs
    )
    last_exec_time_ns = res.exec_time_ns
    last_results = res

    fwd = 0.0
    gold = 0.0
    for r in res.results:
        fwd += float(r["out"][0, 0])
        gold += float(r["out"][0, 1])
    fwd += B * T * c0
    return np.float32(fwd - gold)


# revision 32
# speedup vs baseline: 1.0091x; 1.0091x over previous
"""CRF loss kernel for Trainium2 (8 NeuronCores, data-parallel over batch).

Math: loss = sum_b logZ_b - sum_b gold_b   (lengths unused by the reference).

Forward algorithm in the exp domain:
    P_t = D_t E P_{t-1},  D_t = diag(exp(feats[:, t-1, :])),  E = exp(transitions)
    logZ = ln(estop^T P_T),  estop = exp(transitions[STOP, :])
Run half the time steps forward (P chain) and half backward
(gamma_t = F_t o (E^T gamma_{t+1}), gamma_512 = F_512 o estop), meeting at T/2:
    logZ = ln(beta_256^T P_256),  beta_256 = E^T gamma_257.
Each E application is pre-scaled by exp(-c0) (c0 ~ mean per-step log-growth,
estimated on host).  With that centering the chain magnitude drift stays
within e^{+-8} for this data (measured), well inside bf16/f32 range, so NO
renormalization steps are needed at all.

Gold score on the tensor engine via host-built one-hot matrices, with fp8
DoubleRow matmuls (two 128-row chunks contracted per instruction):
    emit  = trace( sum_chunks OHc^T @ feats_chunk )
    trans = < sum_chunks OHc^T @ OHp , transitions >
with an extra row per example for the STOP transition.
"""

import os
import sys

sys.path.insert(0, "/opt/trn_rl_repo")

import numpy as np
import ml_dtypes

import concourse.bass as bass
import concourse.tile as tile
from concourse import mybir
from concourse.bass_utils import run_bass_kernel_spmd

B, T, K = 512, 512, 128
NCORES = 8
BL = B // NCORES
START, STOP = 126, 127
HALF = T // 2
FCH = 16  # time steps per F chunk
NFCH = HALF // FCH  # chunks per stream
GROWS = 33280  # BL*T + BL stop rows, padded to 130*256
NPAIR = GROWS // 256  # 130 DoubleRow pairs
GJ = 10  # pairs per DMA group
NGDMA = NPAIR // GJ  # 13 dma groups

bf16 = mybir.dt.bfloat16
f32 = mybir.dt.float32
fp8 = mybir.dt.float8e4
NP_BF16 = np.dtype(ml_dtypes.bfloat16)
NP_FP8 = np.dtype(mybir.dt.np(fp8))

_cached = {}


def _fix_multiwait(nc):
    """Walrus here accepts a single sync-wait per instruction; hoist extra
    waits onto single-wait NoOps inserted before the offender.

    Wait choice matters for latency: the wait kept ON the instruction should
    be the one most likely to actually block (a cross-engine data dep), so
    the hoisted NoOps retire early and add no serial hop.  Same-engine
    ge-waits are trivially satisfied (in-order completion, monotone sems)
    and DMA-queue waits are prefetch-slack — hoist those.
    """
    # sem id -> set of engines whose instructions update it
    sem_engines = {}
    for f in nc.m.functions:
        for bb in f.blocks:
            for inst in bb.instructions:
                si = getattr(inst, "sync_info", None)
                if si is None:
                    continue
                for u in si.on_update:
                    uid = getattr(u, "id", None)
                    if uid is not None:
                        sem_engines.setdefault(uid, set()).add(inst.engine)

    n = 0
    for f in nc.m.functions:
        for bb in f.blocks:
            insts = bb.instructions
            out = []
            changed = False
            for inst in insts:
                si = getattr(inst, "sync_info", None)
                if si is not None and len(si.on_wait) > 1:
                    # merge redundant ge-waits on the same semaphore
                    merged = {}
                    rest = []
                    for w in si.on_wait:
                        if getattr(w, "wait_mode", None) == "sem-ge-imm":
                            key = w.id
                            if key in merged:
                                if w.wait_value > merged[key].wait_value:
                                    merged[key] = w
                            else:
                                merged[key] = w
                        else:
                            rest.append(w)
                    waits = list(merged.values()) + rest

                    def prio(w):
                        engs = sem_engines.get(getattr(w, "id", None))
                        if engs is None:
                            return 0  # DMA/external: prefetched, hoist first
                        if engs == {inst.engine}:
                            return 1  # self-engine: trivially satisfied
                        return 2  # cross-engine data dep: keep on inst

                    # a ge-wait on a sem only ever incremented by THIS engine
                    # is satisfied by in-order execution; drop it entirely
                    waits = [
                        w
                        for w in waits
                        if not (
                            prio(w) == 1
                            and getattr(w, "wait_mode", None) == "sem-ge-imm"
                        )
                    ] or waits[-1:]
                    waits.sort(key=prio)
                    if len(waits) == 1:
                        inst.sync_info = mybir.SyncInfo(
                            on_wait=waits, on_update=list(si.on_update)
                        )
                        out.append(inst)
                        continue
                    for j, w in enumerate(waits[:-1]):
                        out.append(
                            mybir.InstNoOp(
                                name=f"{inst.name}-ws{j}",
                                engine=inst.engine,
                                sync_info=mybir.SyncInfo(
                                    on_wait=[w], on_update=[]
                                ),
                                bass_nofuse=True,
                            )
                        )
                        n += 1
                    inst.sync_info = mybir.SyncInfo(
                        on_wait=[waits[-1]], on_update=list(si.on_update)
                    )
                    changed = True
                out.append(inst)
            if changed:
                bb.instructions = out
    return n


def _build_module():
    from contextlib import ExitStack

    nc = bass.Bass("TRN2", target_bir_lowering=False, debug=False)

    def din(name, shape, dt):
        return nc.dram_tensor(name, shape, dt, kind="ExternalInput").ap()

    efwd = din("efwd", [K, K], bf16)  # lhsT for P-chain: exp(trans-c0).T
    ebwd = din("ebwd", [K, K], bf16)  # lhsT for gamma-chain: exp(trans-c0)
    zerosb = din("zerosb", [K, K], bf16)
    estop = din("estop", [K, 1], f32)
    p0 = din("p0", [K, BL], bf16)
    fkb = din("fkb", [K, T, BL], bf16)  # feats, k-major
    grhs = din("grhs", [GROWS, 2 * K], fp8)  # [feats | onehot(prev)] rows
    ohc = din("ohc", [GROWS, K], fp8)  # onehot(cur tag)
    onesb = din("onesb", [K, K], bf16)
    onesf = din("onesf", [K, K], f32)
    ident = din("ident", [K, K], f32)
    transf = din("transf", [K, K], f32)
    out_ap = nc.dram_tensor("out", [1, 2], f32, kind="ExternalOutput").ap()

    # DoubleRow pair layout: pair j = chunks (2j, 2j+1); chunk i is rows
    # [ (g*GJ+j)*256 + i*128 + p ] of the row-major DRAM tensors.
    grhs_g = grhs.rearrange("(g j i p) n -> g p j i n", p=128, i=2, j=GJ)
    ohc_g = ohc.rearrange("(g j i p) k -> g p j i k", p=128, i=2, j=GJ)

    AL = mybir.AluOpType

    with tile.TileContext(nc) as tc:
        with ExitStack() as ctx:
            consts = ctx.enter_context(tc.tile_pool(name="consts", bufs=1))
            state = ctx.enter_context(tc.tile_pool(name="state", bufs=4))
            fraw = ctx.enter_context(tc.tile_pool(name="fraw", bufs=3))
            fexp = ctx.enter_context(tc.tile_pool(name="fexp", bufs=3))
            goldp = ctx.enter_context(tc.tile_pool(name="goldp", bufs=3))
            smalls = ctx.enter_context(tc.tile_pool(name="smalls", bufs=4))
            psf = ctx.enter_context(
                tc.tile_pool(name="psf", bufs=3, space="PSUM")
            )
            psb = ctx.enter_context(
                tc.tile_pool(name="psb", bufs=3, space="PSUM")
            )
            psj = ctx.enter_context(
                tc.tile_pool(name="psj", bufs=1, space="PSUM")
            )
            psacc = ctx.enter_context(
                tc.tile_pool(name="psacc", bufs=1, space="PSUM")
            )

            # ---- chain-critical constants first (single sync DMA queue) ----
            efwd_sb = consts.tile([K, K], bf16)
            nc.sync.dma_start(efwd_sb[:], efwd[:, :])
            ebwd_sb = consts.tile([K, K], bf16)
            estop_sb = consts.tile([K, 1], f32)
            zerosb_sb = consts.tile([K, K], bf16)
            nc.gpsimd.memset(zerosb_sb[:], 0.0)

            def load_start_consts2():
                nc.gpsimd.dma_start(estop_sb[:], estop[:, :])
                nc.sync.dma_start(ebwd_sb[:], ebwd[:, :])
            # junction-only constants are DMA'd after the loop is underway
            onesb_sb = consts.tile([K, K], bf16)
            onesf_sb = consts.tile([K, K], f32)
            ident_sb = consts.tile([K, K], f32)
            transf_sb = consts.tile([K, K], f32)

            def load_tail_consts():
                nc.sync.dma_start(onesb_sb[:], onesb[:, :])
                nc.sync.dma_start(onesf_sb[:], onesf[:, :])
                nc.sync.dma_start(ident_sb[:], ident[:, :])
                nc.sync.dma_start(transf_sb[:], transf[:, :])

            # gold PSUM accumulator: [OHc^T @ feats | OHc^T @ OHp]
            a12 = psacc.tile([K, 2 * K], f32)

            # ---- F chunk machinery ----
            # Irregular chunk lengths: a tiny first chunk gets the chains
            # started ~8us earlier; steady-state chunks are FCH steps.
            CLENS = [4] + [FCH] * 15 + [HALF - 4 - 15 * FCH]
            CSTARTS = [sum(CLENS[:i]) for i in range(len(CLENS))]
            NCH = len(CLENS)
            ftiles = [{}, {}]

            def ensure_fchunk(stream, c):
                if c >= NCH or c in ftiles[stream]:
                    return
                # chunk c covers chain offsets [CSTARTS[c], +CLENS[c]) of
                # stream: fwd feats idx o; bwd feats idx T-1-o
                ln = CLENS[c]
                t0 = (
                    CSTARTS[c]
                    if stream == 0
                    else T - CSTARTS[c] - ln
                )
                raw = fraw.tile([K, ln, BL], bf16, tag=f"raw{stream}")
                nc.sync.dma_start(raw[:], fkb[:, t0 : t0 + ln, :])
                fe = fexp.tile([K, ln, BL], bf16, tag=f"fe{stream}")
                nc.scalar.activation(
                    fe[:], raw[:], mybir.ActivationFunctionType.Exp
                )
                ftiles[stream][c] = fe

            import bisect

            def chunk_of(o):
                return bisect.bisect_right(CSTARTS, o) - 1

            def fslice(stream, fi):
                # fi is a feats index; stream offset o: fwd o=fi, bwd o=T-1-fi
                o = fi if stream == 0 else T - 1 - fi
                c = chunk_of(o)
                fe = ftiles[stream][c]
                off = o - CSTARTS[c]
                if stream == 1:
                    # bwd tile holds feats idx [T-CSTARTS-ln, T-CSTARTS) in
                    # ascending t; offset o counts DOWN from idx T-1
                    off = CLENS[c] - 1 - off
                return fe[:, off, :]

            ensure_fchunk(0, 0)
            ensure_fchunk(1, 0)

            # ---- chain state init ----
            p0_sb = consts.tile([K, BL], bf16)
            nc.gpsimd.dma_start(p0_sb[:], p0[:, :])
            load_start_consts2()
            p_t = state.tile([K, BL], bf16, tag="P")
            nc.vector.tensor_copy(p_t[:], p0_sb[:])
            g_t = state.tile([K, BL], bf16, tag="G")
            # gamma_512 = F(feats idx 511) o estop (per-partition scalar)
            nc.vector.tensor_scalar_mul(g_t[:], fslice(1, T - 1), estop_sb[:])

            # ---- gold machinery (fp8 DoubleRow pairs) ----
            gold_tiles = {}

            def gold_load(g):
                if g >= NGDMA or g in gold_tiles:
                    return
                rh_t = goldp.tile([128, GJ, 2, 2 * K], fp8, tag="rh")
                nc.gpsimd.dma_start(rh_t[:], grhs_g[g])
                oc_t = goldp.tile([128, GJ, 2, K], fp8, tag="oc")
                nc.gpsimd.dma_start(oc_t[:], ohc_g[g])
                gold_tiles[g] = (rh_t, oc_t)

            def gold_pair(pj):
                g, j = divmod(pj, GJ)
                rh_t, oc_t = gold_tiles[g]
                nc.tensor.matmul(
                    a12[:],
                    oc_t[:, j, :, :],
                    rh_t[:, j, :, :],
                    start=False,
                    stop=(pj == NPAIR - 1),
                    perf_mode=mybir.MatmulPerfMode.DoubleRow,
                    skip_group_check=True,
                )

            gold_load(0)
            gold_load(1)

            def pace_mm(src):
                # zero-contribution accumulate into a12: numerically exact
                # (stationary is all zeros), but it (a) reads this round's
                # fresh chain state, and (b) sits in a12's program-order
                # write chain — so the statically-scheduled gold DR matmuls
                # behind it cannot clump ahead of the chain.  Doubles as a
                # PE-pipeline warmer between dependent chain matmuls.
                nc.tensor.matmul(
                    a12[:, 0:BL],
                    zerosb_sb[:],
                    src[:],
                    start=False,
                    stop=False,
                    skip_group_check=True,
                )

            # ---- main loop: 256 rounds, no renorms ----
            # Gold pairs start at round 12 (their first DMA group lands
            # after the chain-critical startup traffic); the leftovers ride
            # the odd rounds near the end.
            GOLD0 = 12
            NINLOOP = (HALF - GOLD0) // 2  # 122 in-loop pairs
            TAIL0 = HALF - 2 * (NPAIR - NINLOOP) + 1  # first tail round
            braw = None
            prev_p, prev_g = p0_sb, p0_sb
            for r in range(HALF):
                # fwd step r+1 (feats idx r)
                praw = psf.tile([K, BL], f32, tag="praw")
                nc.tensor.matmul(
                    praw[:], efwd_sb[:], p_t[:], start=True, stop=True
                )
                # bwd step (feats idx 510-r); at r=255 this matmul IS the
                # junction product beta_256 = E'^T gamma_257
                graw = psb.tile([K, BL], f32, tag="graw")
                nc.tensor.matmul(
                    graw[:], ebwd_sb[:], g_t[:], start=True, stop=True
                )
                # gold pair on even rounds; tail pairs on late odd rounds;
                # warm filler (paced on the PREVIOUS round's state, so it is
                # ready-to-run here and bridges the PE gap while this
                # round's TTs are still in flight) otherwise
                if r == 0:
                    # initialize the full a12 accumulator region to zero
                    # (all later writers use start=False)
                    for h in (0, 1):
                        nc.tensor.matmul(
                            a12[:, 128 * h : 128 * h + 128],
                            zerosb_sb[:],
                            efwd_sb[:],
                            start=True,
                            stop=False,
                            skip_group_check=True,
                        )
                if r % 2 == 0 and GOLD0 <= r < GOLD0 + 2 * NINLOOP:
                    gold_pair((r - GOLD0) // 2)
                elif r % 2 == 1 and r >= TAIL0:
                    gold_pair(NINLOOP + (r - TAIL0) // 2)
                if r < HALF - 2:
                    pace_mm(prev_p)
                # paced filler reading the PREVIOUS round's state: already
                # runnable when it reaches the head of the PE stream, so it
                # bridges the gap while this round's TTs are in flight
                if r < HALF - 2:
                    pace_mm(prev_g)

                p_new = state.tile([K, BL], bf16, tag="P")
                nc.vector.tensor_tensor(
                    out=p_new[:], in0=praw[:], in1=fslice(0, r), op=AL.mult
                )
                prev_p = p_t
                p_t = p_new
                if r < HALF - 1:
                    g_new = state.tile([K, BL], bf16, tag="G")
                    nc.vector.tensor_tensor(
                        out=g_new[:],
                        in0=graw[:],
                        in1=fslice(1, T - 2 - r),
                        op=AL.mult,
                    )
                    prev_g = g_t
                    g_t = g_new
                else:
                    braw = graw

                if r == 2:
                    load_tail_consts()
                # prefetches, early in each chunk/group window
                c_now = chunk_of(r)
                if r == CSTARTS[c_now] + 1:
                    ensure_fchunk(0, c_now + 1)
                    ensure_fchunk(1, c_now + 1)
                    if c_now == 0:
                        ensure_fchunk(0, 2)
                        ensure_fchunk(1, 2)
                if r % 20 == 3:
                    gold_load(r // 20 + 1)
                if r % 20 == 11:
                    gold_load(r // 20 + 2)

            # ---- junction: J_b = sum_k braw[k,b] * P_256[k,b] ----
            jprod = smalls.tile([K, BL], bf16, tag="jprod")
            nc.vector.tensor_tensor(
                out=jprod[:], in0=braw[:], in1=p_t[:], op=AL.mult
            )
            jall_ps = psj.tile([K, BL], f32, tag="zps")
            nc.tensor.matmul(
                jall_ps[:], onesb_sb[:], jprod[:], start=True, stop=True
            )
            lnj = smalls.tile([1, BL], f32, tag="lnj")
            nc.scalar.activation(
                lnj[:], jall_ps[0:1, :], mybir.ActivationFunctionType.Ln
            )
            fwdsum = smalls.tile([1, 1], f32, tag="fwdsum")
            nc.vector.tensor_reduce(
                fwdsum[:], lnj[:], axis=mybir.AxisListType.X, op=AL.add
            )

            # ---- gold finals ----
            junk1 = smalls.tile([K, K], f32, tag="junk1")
            emit_pp = smalls.tile([K, 1], f32, tag="emit_pp")
            nc.vector.scalar_tensor_tensor(
                out=junk1[:],
                in0=a12[:, 0:K],
                scalar=1.0,
                in1=ident_sb[:],
                op0=AL.mult,
                op1=AL.mult,
                accum_out=emit_pp[:],
            )
            junk2 = smalls.tile([K, K], f32, tag="junk2")
            tr_pp = smalls.tile([K, 1], f32, tag="tr_pp")
            nc.vector.scalar_tensor_tensor(
                out=junk2[:],
                in0=a12[:, K : 2 * K],
                scalar=1.0,
                in1=transf_sb[:],
                op0=AL.mult,
                op1=AL.mult,
                accum_out=tr_pp[:],
            )
            gold_pp = smalls.tile([K, 1], bf16, tag="gold_pp")
            with nc.allow_low_precision(
                reason="per-tag gold partial sums; 0.4% rounding is far "
                "inside the 2e-2 tolerance"
            ):
                nc.vector.tensor_add(gold_pp[:], emit_pp[:], tr_pp[:])
            gall_ps = psj.tile([K, 1], f32, tag="zps")
            nc.tensor.matmul(
                gall_ps[:], onesb_sb[:], gold_pp[:], start=True, stop=True
            )

            # ---- output ----
            res = smalls.tile([1, 2], f32, tag="res")
            nc.vector.tensor_copy(res[:, 0:1], fwdsum[:])
            nc.vector.tensor_copy(res[:, 1:2], gall_ps[0:1, :])
            nc.sync.dma_start(out_ap[:, :], res[:])

    _fix_multiwait(nc)
    return nc


def _estimate_c0(feats, transitions):
    """Mean per-step log-growth of the forward recursion, from a few batches."""
    nb = 4
    E = np.exp(transitions.astype(np.float64))
    P = np.zeros((K, nb))
    P[START, :] = 1.0
    tot = 0.0
    for t in range(T):
        P = E @ P
        P = P * np.exp(feats[:nb, t, :].astype(np.float64)).T
        s = P.sum(axis=0)
        tot += np.log(s).mean()
        P /= s
    return tot / T


def _host_prep(feats, tags, transitions):
    c0 = _estimate_c0(feats, transitions)
    ep = np.exp(transitions.astype(np.float64) - c0)
    efwd_np = np.ascontiguousarray(ep.T).astype(NP_BF16)
    ebwd_np = np.ascontiguousarray(ep).astype(NP_BF16)
    estop_np = np.exp(transitions[STOP, :].astype(np.float64)).astype(
        np.float32
    )[:, None]
    ident_np = np.eye(K, dtype=np.float32)
    zerosb_np = np.zeros((K, K), dtype=NP_BF16)
    onesb_np = np.ones((K, K), dtype=NP_BF16)
    onesf_np = np.ones((K, K), dtype=np.float32)
    transf_np = transitions.astype(np.float32)
    p0_np = np.zeros((K, BL), dtype=NP_BF16)
    p0_np[START, :] = 1.0

    in_maps = []
    for c in range(NCORES):
        b0 = c * BL
        fc = feats[b0 : b0 + BL]  # [BL, T, K] f32
        tg = tags[b0 : b0 + BL].astype(np.int32)  # [BL, T]

        fkb_np = np.ascontiguousarray(fc.transpose(2, 1, 0)).astype(NP_BF16)

        nrow = BL * T
        grhs_np = np.zeros((GROWS, 2 * K), dtype=NP_FP8)
        grhs_np[:nrow, :K] = fc.reshape(nrow, K).astype(NP_FP8)
        ohc_np = np.zeros((GROWS, K), dtype=NP_FP8)
        rows = np.arange(nrow)
        ohc_np[rows, tg.reshape(nrow)] = 1.0
        prev = np.concatenate(
            [np.full((BL, 1), START, np.int32), tg[:, :-1]], axis=1
        )
        grhs_np[rows, K + prev.reshape(nrow)] = 1.0
        # stop rows: trans[STOP, tag_last] per example
        srows = nrow + np.arange(BL)
        ohc_np[srows, STOP] = 1.0
        grhs_np[srows, K + tg[:, -1]] = 1.0

        in_maps.append(
            {
                "efwd": efwd_np,
                "ebwd": ebwd_np,
                "zerosb": zerosb_np,
                "estop": estop_np,
                "p0": p0_np,
                "fkb": fkb_np,
                "grhs": grhs_np,
                "ohc": ohc_np,
                "ident": ident_np,
                "onesb": onesb_np,
                "onesf": onesf_np,
                "transf": transf_np,
            }
        )
    return in_maps, c0


last_exec_time_ns = None
last_results = None


def kernel(feats, tags, lengths, transitions):
    global last_exec_time_ns, last_results
    feats = np.asarray(feats, dtype=np.float32)
    tags = np.asarray(tags)
    transitions = np.asarray(transitions, dtype=np.float32)

    if "nc" not in _cached:
        _cached["nc"] = _build_module()
    nc = _cached["nc"]

    in_maps, c0 = _host_prep(feats, tags, transitions)

    trace = bool(int(os.environ.get("BASS_CRF_TRACE", "0")))
    kwargs = {}
    if trace:
        kwargs = {
            "trace": True,
            "tmpdir": os.environ.get("BASS_CRF_TMPDIR", "/tmp/crf_trace"),
        }
    res = run_bass_kernel_spmd(
        nc, in_maps, core_ids=list(range(NCORES)), **kwargs
    )
    last_exec_time_ns = res.exec_time_ns
    last_results = res

    fwd = 0.0
    gold = 0.0
    for r in res.results:
        fwd += float(r["out"][0, 0])
        gold += float(r["out"][0, 1])
    fwd += B * T * c0
    return np.float32(fwd - gold)
